# revision 2
# baseline (speedup 1.0000x reference)
"""Trainium2 Bass kernel for a 2-layer GRU (H=256) + FC head — v3.

Problem: x [512, 1024, 1] -> 2-layer GRU(hidden 256, batch_first) -> FC(256->1)
on the last timestep's hidden state. Output [512, 1].

v3 strategy (data-parallel over 8 NeuronCores, B=64 per core):
- Batch-stationary fp16 matmuls (v1's layout): h kept as [batch, hidden],
  layers stacked on the partition dim (L0 rows 0:64, L1 rows 64:128). PE
  moving streams (gate dim, 512/256 rows) run at 1 cycle/row (vs 4 for
  fp32) and hide the 64-col stationary loads (a weights-stationary layout
  thrashes LDWEIGHTS: ~107ns fixed per load).
- L1 lags L0 by TWO timesteps: its input-side wih1 matmuls read s0 from
  two iterations back, so they run off the recurrent critical path
  (emitted one iteration early, as real PE filler work).
- One K=3 shared-stationary aug matmul per PSUM bank carries both layers'
  biases + L0's scalar-input projection (x3 stationary = [x_t masked to
  L0 rows; ones masked to L0; ones masked to L1]).
- Gate math fp16 in two 128-col half-chains (half 0's tanh/update/
  transpose/copy overlaps half 1's DVE ops); sigmoid split into r and z
  instructions so the n-path starts earlier; the two h' transposes get
  separate PSUM banks so the s-copies don't serialize against each other
  (Tile's bank-overlap tracker).
- Dep-free warming matmuls keep the PE's HAM clock-gate at 2.4 GHz
  through the gate-math tail (idle windows drop it to 1.2 GHz).
"""

import numpy as np

H = 256
B_CORE = 64
N_CORES = 8
T_FULL = 1024
XCHUNK = 64   # timesteps per x3 DMA chunk
N_FILL_A = 3  # PE warming mms between bank-front and the gate tail

_BUILD_CACHE = {}


def _build(T):
    """Build + compile the per-core Bass program for sequence length T."""
    from contextlib import ExitStack

    import concourse.bass as bass
    import concourse.tile as tile
    import concourse.mybir as mybir
    from concourse import bacc
    from concourse.bass import _add_dep_helper as _dep

    f32 = mybir.dt.float32
    f16 = mybir.dt.float16
    AF = mybir.ActivationFunctionType

    n_iter = T + 2  # L1 lags by two steps; final two iters finish L1
    n_xchunks = (n_iter + XCHUNK - 1) // XCHUNK
    TPAD = n_xchunks * XCHUNK

    nc = bacc.Bacc(
        "TRN2", target_bir_lowering=False, debug=False, num_devices=N_CORES
    )

    x3_d = nc.dram_tensor("x3", [3, TPAD, 128], f16, kind="ExternalInput")
    whh0_d = nc.dram_tensor("whh0T", [128, 2, 3 * H], f16, kind="ExternalInput")
    whh1_d = nc.dram_tensor("whh1T", [128, 2, 3 * H], f16, kind="ExternalInput")
    wih1_d = nc.dram_tensor("wih1T", [128, 2, 3 * H], f16, kind="ExternalInput")
    augw_d = nc.dram_tensor("augw", [3, 1024], f16, kind="ExternalInput")
    wfc_d = nc.dram_tensor("wfc", [128, 2], f16, kind="ExternalInput")
    bfc_d = nc.dram_tensor("bfc", [1, 1], f32, kind="ExternalInput")
    ident_d = nc.dram_tensor("ident", [128, 128], f16, kind="ExternalInput")
    y_d = nc.dram_tensor("y", [1, B_CORE], f32, kind="ExternalOutput")

    with tile.TileContext(nc) as tc, ExitStack() as ctx:
        const = ctx.enter_context(tc.tile_pool(name="const", bufs=1))
        xq = ctx.enter_context(tc.tile_pool(name="xq", bufs=2))
        hpool = ctx.enter_context(tc.tile_pool(name="hpool", bufs=3))
        spool = ctx.enter_context(tc.tile_pool(name="spool", bufs=4))
        gates = ctx.enter_context(tc.tile_pool(name="gates", bufs=2))
        psA = ctx.enter_context(tc.tile_pool(name="psA", bufs=2, space="PSUM"))
        psB = ctx.enter_context(tc.tile_pool(name="psB", bufs=2, space="PSUM"))
        psC0 = ctx.enter_context(tc.tile_pool(name="psC0", bufs=1, space="PSUM"))
        psC1 = ctx.enter_context(tc.tile_pool(name="psC1", bufs=1, space="PSUM"))
        psFC = ctx.enter_context(tc.tile_pool(name="psFC", bufs=1, space="PSUM"))
        psW = ctx.enter_context(tc.tile_pool(name="psW", bufs=1, space="PSUM"))

        whh0 = const.tile([128, 2, 3 * H], f16)
        whh1 = const.tile([128, 2, 3 * H], f16)
        wih1 = const.tile([128, 2, 3 * H], f16)
        augw = const.tile([3, 1024], f16)
        wfc = const.tile([128, 2], f16)
        bfc = const.tile([1, 1], f32)
        ident = const.tile([128, 128], f16)
        for sb, dr in [(whh0, whh0_d), (whh1, whh1_d), (wih1, wih1_d),
                       (augw, augw_d), (wfc, wfc_d), (bfc, bfc_d),
                       (ident, ident_d)]:
            nc.sync.dma_start(out=sb, in_=dr.ap())

        h_prev = hpool.tile([128, H], f16, tag="h")
        nc.vector.memset(h_prev, 0.0)
        s_m1 = spool.tile([128, 2, 128], f16, tag="s")  # s from iter t-1
        nc.vector.memset(s_m1, 0.0)

        mm = nc.tensor.matmul
        xchunks = {}

        def get_x3(t):
            tq = t // XCHUNK
            if tq not in xchunks:
                xc = xq.tile([3, XCHUNK, 128], f16, tag="xc", name="xc")
                nc.sync.dma_start(
                    out=xc, in_=x3_d.ap()[:, tq * XCHUNK: (tq + 1) * XCHUNK, :])
                xchunks[tq] = xc
                xchunks.pop(tq - 2, None)
            return xchunks[tq][:, t % XCHUNK, :]

        def chain(last_box, *args, **kw):
            # Linear same-engine ordering per PSUM bank (Tile does not
            # WAW-order PSUM-accumulate writes).
            m_ = mm(*args, skip_group_check=True, **kw)
            if last_box[0] is not None:
                _dep(m_.ins, last_box[0].ins, sync=False,
                     reason="psum accumulation order")
            last_box[0] = m_
            return m_

        def emit_bank_front(t, sB):
            """Open step t's PSUM banks: shared-stationary aug mms plus L1's
            input-side wih1 matmuls (L1 step t-2 reads s0 from iteration t-2,
            already available — chain-independent PE work). sB = s(t-2)."""
            x3 = get_x3(t)
            rz_ps = psA.tile([128, 2 * H], f32, tag="rz", name="rz_ps")
            n_ps = psB.tile([128, 2 * H], f32, tag="nb", name="n_ps")
            rzl = [None]
            nl = [None]
            chain(rzl, rz_ps, x3, augw[:, 0:512], start=True, stop=False)
            chain(nl, n_ps, x3, augw[:, 512:1024], start=True, stop=False)
            if 2 <= t:
                chain(rzl, rz_ps[64:128, :], sB[:, 0, 0:64],
                      wih1[:, 0, 0:512], start=False, stop=False)
                chain(rzl, rz_ps[64:128, :], sB[:, 1, 0:64],
                      wih1[:, 1, 0:512], start=False, stop=False)
                chain(nl, n_ps[64:128, H:2 * H], sB[:, 0, 0:64],
                      wih1[:, 0, 512:768], start=False, stop=False)
                chain(nl, n_ps[64:128, H:2 * H], sB[:, 1, 0:64],
                      wih1[:, 1, 512:768], start=False, stop=False)
            return rz_ps, n_ps, rzl, nl

        def warm(n):
            for _ in range(n):
                wtile = psW.tile([128, 512], f32, tag="warm", name="warm")
                mm(wtile, ident, whh0[:, 0, 0:512], start=True, stop=True,
                   skip_group_check=True)

        cur = emit_bank_front(0, s_m1)

        for t in range(n_iter):
            rz_ps, n_ps, rzl, nl = cur

            # --- chain matmuls: rz bank first (gates sigmoid), k0 chunks
            # before k1 (s chunk 0's copy lands first).
            if t < T:
                chain(rzl, rz_ps[0:64, :], s_m1[:, 0, 0:64],
                      whh0[:, 0, 0:512], start=False, stop=False)
            if 2 <= t:
                chain(rzl, rz_ps[64:128, :], s_m1[:, 0, 64:128],
                      whh1[:, 0, 0:512], start=False, stop=False)
            if t < T:
                chain(rzl, rz_ps[0:64, :], s_m1[:, 1, 0:64],
                      whh0[:, 1, 0:512], start=False, stop=True)
            if 2 <= t:
                chain(rzl, rz_ps[64:128, :], s_m1[:, 1, 64:128],
                      whh1[:, 1, 0:512], start=False, stop=True)
            if t < T:
                chain(nl, n_ps[0:64, 0:H], s_m1[:, 0, 0:64],
                      whh0[:, 0, 512:768], start=False, stop=False)
            if 2 <= t:
                chain(nl, n_ps[64:128, 0:H], s_m1[:, 0, 64:128],
                      whh1[:, 0, 512:768], start=False, stop=False)
            if t < T:
                chain(nl, n_ps[0:64, 0:H], s_m1[:, 1, 0:64],
                      whh0[:, 1, 512:768], start=False, stop=True)
            if 2 <= t:
                chain(nl, n_ps[64:128, 0:H], s_m1[:, 1, 64:128],
                      whh1[:, 1, 512:768], start=False, stop=True)

            # Next step's bank front: aug + wih1 mms are real off-chain PE
            # work that fills the gate-math phase.
            nxt = emit_bank_front(t + 1, s_m1) if t + 1 < n_iter else None
            warm(N_FILL_A)

            # --- gates, two independent 128-col half-chains; sigmoid split
            # into r and z so the n-path starts earlier.
            rz_sb = gates.tile([128, 2 * H], f16, tag="rz_sb")
            nc.scalar.activation(rz_sb[:, 0:H], rz_ps[:, 0:H], AF.Sigmoid)
            nc.scalar.activation(rz_sb[:, H:2 * H], rz_ps[:, H:2 * H],
                                 AF.Sigmoid)
            t1 = gates.tile([128, H], f32, tag="t1")
            t2 = gates.tile([128, H], f16, tag="t2")
            n_sb = gates.tile([128, H], f16, tag="n_sb")
            d_sb = gates.tile([128, H], f16, tag="d_sb")
            e_sb = gates.tile([128, H], f16, tag="e_sb")
            h_new = hpool.tile([128, H], f16, tag="h")
            s_new = spool.tile([128, 2, 128], f16, tag="s")
            for hh in range(2):
                c = slice(128 * hh, 128 * (hh + 1))
                z = slice(256 + 128 * hh, 384 + 128 * hh)
                nc.vector.tensor_mul(t1[:, c], rz_sb[:, c], n_ps[:, c])
                nc.vector.tensor_add(t2[:, c], t1[:, c], n_ps[:, z])
                nc.scalar.activation(n_sb[:, c], t2[:, c], AF.Tanh)
                nc.vector.tensor_sub(d_sb[:, c], h_prev[:, c], n_sb[:, c])
                nc.vector.tensor_mul(e_sb[:, c], rz_sb[:, z], d_sb[:, c])
                nc.vector.tensor_add(h_new[:, c], n_sb[:, c], e_sb[:, c])
                if t < 2:
                    # L1's steps "-2"/"-1" are junk; true initial state is 0.
                    nc.vector.memset(h_new[64:128, c], 0.0)
                trp = (psC0.tile([128, 128], f16, tag="tr0", name="tr0")
                       if hh == 0 else
                       psC1.tile([128, 128], f16, tag="tr1", name="tr1"))
                nc.tensor.transpose(trp, h_new[:, c], ident)
                if hh == 0:
                    # chain-critical copy on the faster DVE path
                    nc.vector.tensor_copy(s_new[:, 0, :], trp)
                else:
                    nc.scalar.activation(s_new[:, 1, :], trp, AF.Copy)
                if t < 2:
                    nc.vector.memset(s_new[:, hh, 64:128], 0.0)

            h_prev = h_new
            s_m1 = s_new
            cur = nxt

        # --- FC head: y = h1(T-1) @ w_fc^T + b_fc, using S = h^T chunks
        fc_ps = psFC.tile([1, B_CORE], f32, tag="fc")
        mm(fc_ps, wfc[:, 0:1], s_m1[:, 0, 64:128], start=True, stop=False)
        mm(fc_ps, wfc[:, 1:2], s_m1[:, 1, 64:128], start=False, stop=True)
        y_sb = const.tile([1, B_CORE], f32)
        nc.scalar.activation(y_sb, fc_ps, AF.Identity, bias=bfc[0:1, 0:1])
        nc.sync.dma_start(out=y_d.ap(), in_=y_sb)

    nc.compile()
    return nc


def _get_nc(T):
    if T not in _BUILD_CACHE:
        _BUILD_CACHE[T] = _build(T)
    return _BUILD_CACHE[T]


def _prep_weight_inputs(w_ih_l0, w_hh_l0, b_ih_l0, b_hh_l0,
                        w_ih_l1, w_hh_l1, b_ih_l1, b_hh_l1, w_fc, b_fc):
    f = np.float16

    def wT(w):
        # w [768, 256] -> [p, k, g] = w[g, k*128+p]
        return np.ascontiguousarray(
            w.T.reshape(2, 128, 3 * H).transpose(1, 0, 2), dtype=f)

    # aug moving weights [3, 1024]:
    #   cols 0:512    rz bank: k0=w_ih_l0_rz, k1=(b_ih+b_hh)_l0_rz,
    #                 k2=(b_ih+b_hh)_l1_rz
    #   cols 512:768  gh_n:    k1=b_hh_l0_n, k2=b_hh_l1_n
    #   cols 768:1024 gx_n:    k0=w_ih_l0_n, k1=b_ih_l0_n, k2=b_ih_l1_n
    augw = np.zeros((3, 1024), np.float32)
    augw[0, 0:512] = w_ih_l0[0:512, 0]
    augw[1, 0:512] = b_ih_l0[0:512] + b_hh_l0[0:512]
    augw[2, 0:512] = b_ih_l1[0:512] + b_hh_l1[0:512]
    augw[1, 512:768] = b_hh_l0[512:768]
    augw[2, 512:768] = b_hh_l1[512:768]
    augw[0, 768:1024] = w_ih_l0[512:768, 0]
    augw[1, 768:1024] = b_ih_l0[512:768]
    augw[2, 768:1024] = b_ih_l1[512:768]

    return {
        "whh0T": wT(w_hh_l0), "whh1T": wT(w_hh_l1), "wih1T": wT(w_ih_l1),
        "augw": augw.astype(f),
        "wfc": np.ascontiguousarray(w_fc.reshape(2, 128).T, dtype=f),
        "bfc": np.asarray(b_fc, np.float32).reshape(1, 1),
        "ident": np.eye(128, dtype=f),
    }


def _prep_x_core(x_core, T):
    """x_core [B_CORE, T, 1] -> x3 [3, TPAD, 128] fp16 aug stationaries.

    x3[0, t, b] = x[b, t] for b<64 else 0 (L0 input row)
    x3[1, t, b] = 1 for b<64 else 0       (L0 bias row)
    x3[2, t, b] = 1 for b>=64 else 0      (L1 bias row)
    """
    n_iter = T + 2
    n_xchunks = (n_iter + XCHUNK - 1) // XCHUNK
    TPAD = n_xchunks * XCHUNK
    xa = np.zeros((3, TPAD, 128), np.float32)
    xa[0, :T, 0:B_CORE] = x_core[:, :, 0].T
    xa[1, :, 0:B_CORE] = 1.0
    xa[2, :, B_CORE:128] = 1.0
    return xa.astype(np.float16)


def run(inputs, T, trace=False):
    """Run the sharded kernel; returns ([B,1] output, BassKernelResults)."""
    from concourse import bass_utils

    x = np.asarray(inputs["x"], np.float32)
    B = x.shape[0]
    assert B == N_CORES * B_CORE and x.shape[1] == T
    nc = _get_nc(T)
    wmap = _prep_weight_inputs(
        *(np.asarray(inputs[k], np.float32) for k in (
            "w_ih_l0", "w_hh_l0", "b_ih_l0", "b_hh_l0",
            "w_ih_l1", "w_hh_l1", "b_ih_l1", "b_hh_l1", "w_fc", "b_fc"))
    )
    in_maps = []
    for c in range(N_CORES):
        m = dict(wmap)
        m["x3"] = _prep_x_core(x[c * B_CORE: (c + 1) * B_CORE], T)
        in_maps.append(m)
    res = bass_utils.run_bass_kernel_spmd(
        nc, in_maps, core_ids=list(range(N_CORES)), trace=trace
    )
    y = np.concatenate(
        [res.results[c]["y"].reshape(B_CORE, 1) for c in range(N_CORES)], axis=0
    )
    return y.astype(np.float32), res


def kernel(**inputs):
    y, _ = run(inputs, T_FULL, trace=False)
    return y
